# revision 2
# baseline (speedup 1.0000x reference)
"""CenterLoss forward on 8 Trainium2 NeuronCores (Bass/Tile).

loss = mean_b ||features[b] - centers[labels[b]]||^2  (LAMBDA_C = 1.0)

Strategy — CLASS-RANGE sharding (the loss is a permutation-invariant sum
over examples, so any example->core routing is a valid sharding):
  - Core k owns classes [12500k, 12500(k+1)). The host routes each example
    to the core that owns its label and ships that core only its 12500-row
    slice of the centers table. Local class indices fit int16, unlocking the
    gpsimd `dma_gather` SWDGE instruction (1024 rows max per instruction).
  - Trace-driven layout (v2):
      * `load_library(mlp)` is issued first so the one-time Q7 IRAM ucode
        load (~6-14us) starts as early as possible; a tiny warmup gather
        then absorbs dispatch so real gathers stream immediately after.
      * Gather descriptor generation costs ~6.1ns/row on a Q7 core and the
        4 SWDGE queues run on 4 cores concurrently -> chunk sizes are
        balanced so every queue gets ~nrb/4 blocks (issue-order lane c+1
        locks chunk c to queue ((c+1)%8)%4).
      * Data ships as fp8 e4m3 (tolerance gate 2e-2; fp8 quantization bias
        ~1.3e-3), halving HBM traffic vs bf16: ~2.2MB features + ~2.2MB
        gathered centers per core. One consolidated feature DMA.
      * Per chunk: DVE subtract (fp8 in, bf16 out); the square+reduce runs
        on ACT (activation Square + accum) for early chunks and on DVE (a
        fused custom-DVE multiply+accumulate pass) for late chunks so the
        two engines split the post-gather work.
  - Pad rows use local class 0 with the pad feature row set to that exact
    (quantized) center row, so they contribute exactly 0 to the sum.
  - Host sums the 8 partial scalars and divides by the batch size.
"""

import ml_dtypes
import numpy as np

import concourse.bacc as bacc
import concourse.mybir as mybir
import concourse.tile as tile
from concourse import library_config
from concourse.bass_utils import run_bass_kernel_spmd
from concourse.dve_ops import TENSOR_TENSOR_REDUCE

NCORES = 8
BATCH = 65536
FEAT_DIM = 256
NUM_CLASSES = 100000
CSHARD = NUM_CLASSES // NCORES  # 12500 classes per core
LAMBDA_C = 1.0
P = 128

USE_FP8 = True
USE_BF16 = not USE_FP8  # kept for test.py compatibility
if USE_FP8:
    _dt = mybir.dt.float8e4
    _np_dt = ml_dtypes.float8_e4m3
else:
    _dt = mybir.dt.bfloat16
    _np_dt = ml_dtypes.bfloat16
_f32 = mybir.dt.float32
_bf16 = mybir.dt.bfloat16

NQ = 4  # SWDGE queues (ucode max)
MAXBLK = 8  # 1024-index cap per dma_gather


def _chunks(nrb):
    """Split nrb 128-row blocks into gather chunks balanced across the 4
    SWDGE queues. Chunk c is locked to queue ((c+1)%8)%4 by Tile's
    issue-order sem-lane assignment (warmup gather is issue 0), so sizes
    are chosen per round-robin position: each queue ends up with
    ~nrb/4 blocks, max 8 blocks (1024 indices) per instruction."""
    quota = [nrb // NQ] * NQ
    for i in range(nrb % NQ):
        quota[i] += 1
    out = []
    b0 = 0
    while any(quota):
        for pos in range(NQ):
            cb = min(MAXBLK, quota[pos])
            if cb <= 0:
                continue
            out.append((b0, cb))
            quota[pos] -= cb
            b0 += cb
    assert b0 == nrb
    return out


def _build(nrb):
    nc = bacc.Bacc(
        "TRN2",
        target_bir_lowering=False,
        debug=False,
        num_devices=NCORES,
        enable_asserts=False,
        # 3x the default SWDGE descriptor-ring carveout so several 1024-row
        # gathers can be in flight while the next one's descriptors generate.
        dynamic_dma_scratch_size=49152,
        num_swdge_queues=NQ,
    )
    feat_d = nc.dram_tensor("features", [P, nrb, FEAT_DIM], _dt, kind="ExternalInput")
    lab_d = nc.dram_tensor("labels", [P, nrb * 8], mybir.dt.int16, kind="ExternalInput")
    cent_d = nc.dram_tensor("centers", [CSHARD, FEAT_DIM], _dt, kind="ExternalInput")
    out_d = nc.dram_tensor("partial", [1, 1], _f32, kind="ExternalOutput")

    chunks = _chunks(nrb)
    nch = len(chunks)
    # Number of leading chunks whose square+reduce runs on ACT; the rest run
    # as a fused multiply+accumulate on DVE. Balances ACT (~2.0us/8-block)
    # against DVE sub-for-all (~1.2us/8-block) + late squares.
    n_act = max(0, nch - 4)

    with tile.TileContext(nc) as tc:
        with (
            tc.tile_pool(name="big", bufs=1) as big,
            tc.tile_pool(name="sc", bufs=2) as sc,
            tc.tile_pool(name="ps", bufs=1, space="PSUM") as ps,
        ):
            # Start the Q7 ucode IRAM load as early as possible.
            nc.gpsimd.load_library(library_config.mlp)

            # Gather indices for the whole shard, wrapped [16, nr/16] and
            # replicated to 128 partitions (dma_gather's expected layout).
            lab = big.tile([P, nrb * 8], mybir.dt.int16)
            nc.sync.dma_start(out=lab[:], in_=lab_d.ap())

            # Warmup gather: absorbs ucode dispatch right after the library
            # load so the real gathers stream immediately.
            warm_idx = big.tile([P, 1], mybir.dt.int16)
            nc.vector.memset(warm_idx[:], 0)
            warm_out = big.tile([P, 1, FEAT_DIM], _dt)
            nc.gpsimd.dma_gather(
                warm_out[:], cent_d.ap(), warm_idx[:], 16, 16, FEAT_DIM
            )

            ones = big.tile([P, 1], _f32)
            nc.vector.memset(ones[:], 1.0)

            # Whole shard stays resident, so gathers and the feature DMA
            # never wait on buffer recycling.
            feat = big.tile([P, nrb, FEAT_DIM], _dt)
            cent = big.tile([P, nrb, FEAT_DIM], _dt)
            acc = big.tile([P, nch], _f32)

            for c, (b0, cb) in enumerate(chunks):
                # One SWDGE instruction gathers cb*128 center rows; row i
                # lands at [i%128, i//128, :], matching the host's feature
                # wrap layout.
                nc.gpsimd.dma_gather(
                    cent[:, b0 : b0 + cb, :],
                    cent_d.ap(),
                    lab[:, b0 * 8 : (b0 + cb) * 8],
                    cb * P,
                    cb * P,
                    FEAT_DIM,
                    queue_num=((c + 1) % 8) % 4,
                )

            # One consolidated feature DMA (fp8: ~2.2MB), issued after the
            # gathers so its SDMA traffic defers to the ucode load + gathers.
            nc.sync.dma_start(out=feat[:], in_=feat_d.ap())

            for c, (b0, cb) in enumerate(chunks):
                diff_t = sc.tile([P, cb, FEAT_DIM], _bf16, tag=f"diff{cb}")
                nc.vector.tensor_tensor(
                    out=diff_t[:],
                    in0=feat[:, b0 : b0 + cb, :],
                    in1=cent[:, b0 : b0 + cb, :],
                    op=mybir.AluOpType.subtract,
                )
                if c < n_act:
                    # Square + per-partition sum on the ACT engine in one
                    # pass; sq is a required but dead output.
                    sq_t = sc.tile([P, cb, FEAT_DIM], _bf16, tag=f"sq{cb}")
                    nc.scalar.activation(
                        out=sq_t[:],
                        in_=diff_t[:],
                        func=mybir.ActivationFunctionType.Square,
                        accum_out=acc[:, c : c + 1],
                    )
                else:
                    # Fused multiply+accumulate on DVE: acc = sum(diff*diff).
                    sq_t = sc.tile([P, cb, FEAT_DIM], _bf16, tag=f"sq{cb}")
                    nc.vector._custom_dve(
                        TENSOR_TENSOR_REDUCE,
                        out=sq_t[:],
                        in0=diff_t[:],
                        in1=diff_t[:],
                        s0=0.0,
                        s1=1.0,
                        accum_out=acc[:, c : c + 1],
                    )

            # acc [128, nch] -> [128, 1] -> [1, 1] -> HBM
            acc1 = big.tile([P, 1], _f32)
            nc.vector.reduce_sum(out=acc1[:], in_=acc[:], axis=mybir.AxisListType.X)
            res_ps = ps.tile([1, 1], _f32)
            nc.tensor.matmul(
                out=res_ps[:], lhsT=acc1[:], rhs=ones[:], start=True, stop=True
            )
            res_sb = big.tile([1, 1], _f32)
            nc.vector.reduce_sum(out=res_sb[:], in_=res_ps[:], axis=mybir.AxisListType.X)
            nc.sync.dma_start(out=out_d.ap(), in_=res_sb[:])

    nc.compile()
    return nc


_nc_cache = {}


def _get_nc(nrb):
    if nrb not in _nc_cache:
        _nc_cache[nrb] = _build(nrb)
    return _nc_cache[nrb]


def _make_in_maps(features, labels, centers):
    features = np.ascontiguousarray(np.asarray(features, dtype=np.float32))
    labels = np.ascontiguousarray(np.asarray(labels)).astype(np.int64)
    centers = np.ascontiguousarray(np.asarray(centers, dtype=np.float32))
    assert features.shape == (BATCH, FEAT_DIM)
    assert labels.shape == (BATCH,)
    assert centers.shape == (NUM_CLASSES, FEAT_DIM)

    bucket = labels // CSHARD
    order = np.argsort(bucket, kind="stable")
    counts = np.bincount(bucket, minlength=NCORES)
    # Blocks of 128 rows; at least NQ so every queue gets work.
    nrb = max(NQ, -(-int(counts.max()) // P))
    nr = nrb * P

    cent_np = centers.astype(_np_dt)
    in_maps = []
    pos = 0
    for k in range(NCORES):
        n = int(counts[k])
        idx = order[pos : pos + n]
        pos += n
        cshard = cent_np[k * CSHARD : (k + 1) * CSHARD]
        feat_k = np.empty((nr, FEAT_DIM), dtype=_np_dt)
        feat_k[:n] = features[idx].astype(_np_dt)
        # Pad rows: local class 0 with its exact center row -> diff == 0.
        feat_k[n:] = cshard[0]
        loc = np.zeros((nr,), dtype=np.int16)
        loc[:n] = (labels[idx] - k * CSHARD).astype(np.int16)
        # dma_gather index layout: index i at [i%16, i//16], replicated to
        # all 128 partitions.
        lab16 = np.ascontiguousarray(
            np.tile(loc.reshape(nr // 16, 16).T, (P // 16, 1))
        )
        # Row i -> partition i%128, block i//128 (matches gather output).
        featw = np.ascontiguousarray(
            feat_k.reshape(nrb, P, FEAT_DIM).transpose(1, 0, 2)
        )
        in_maps.append({"features": featw, "labels": lab16, "centers": cshard})
    return in_maps, nrb


def _reduce_results(results):
    total = sum(float(r["partial"][0, 0]) for r in results)
    return np.float32(LAMBDA_C * total / BATCH)


def kernel(features: np.ndarray, labels: np.ndarray, centers: np.ndarray):
    in_maps, nrb = _make_in_maps(features, labels, centers)
    res = run_bass_kernel_spmd(_get_nc(nrb), in_maps, core_ids=list(range(NCORES)))
    return _reduce_results(res.results)
